# revision 6
# baseline (speedup 1.0000x reference)
"""ChebyKAN layer on 8 Trainium2 NeuronCores.

y[b,o] = sum_{i,d} T_d(tanh(x[b,i])) * coeffs[i,o,d]

T_d computed via the Chebyshev recurrence (T_0=1, T_1=t, T_d=2t*T_{d-1}-T_{d-2})
on ACT/DVE engines directly into [i-partition, batch] SBUF tiles; the einsum is
a (B x 9216) @ (9216 x 1024) matmul done in fp32r (full-rate, ~FP22 precision),
accumulated in PSUM over all 72 contraction blocks.

Sharding: data-parallel over batch (2048 rows/core), coeffs replicated.
"""

import numpy as np
import concourse.mybir as mybir
import concourse.tile as tile
from concourse import bacc
from concourse.bass_utils import run_bass_kernel_spmd

B, I, O, D1 = 16384, 1024, 1024, 9
CORES = 8
BC = B // CORES            # 2048 batch rows per core
P = 128
MACRO = 1024               # batch rows per generation
OH = 512                   # output cols per generation
N_M = BC // MACRO          # 2
N_OH = O // OH             # 2
IB = I // P                # 8 i-blocks
BT = MACRO // P            # 8 batch subtiles per macro

F32 = mybir.dt.float32
F32R = mybir.dt.float32r
AF = mybir.ActivationFunctionType
OP = mybir.AluOpType

_CACHE = {}
_last_in_maps = None


def _emit(nc, xp, tp, wp, c2p, op_, pp, xt_d, c2_d, y_d, ones, rep):
    t_tiles = {}
    for m in range(N_M):
        for oh in range(N_OH):
            psum = [
                pp.tile([P, OH], F32, tag=f"ps{bt}", name=f"ps{bt}_{rep}_{m}_{oh}")
                for bt in range(BT)
            ]
            win = {}
            for d in range(D1):
                if d == 1 and oh == 0:
                    for ib in range(IB):
                        xt = xp.tile([P, MACRO], F32, tag="xt", name=f"xt{rep}_{m}_{ib}")
                        nc.sync.dma_start(
                            xt[:],
                            xt_d[ib * P:(ib + 1) * P, m * MACRO:(m + 1) * MACRO],
                        )
                        t = tp.tile(
                            [P, MACRO], F32R, tag=f"t{ib}", name=f"t{ib}_{rep}_{m}"
                        )
                        nc.scalar.activation(t[:], xt[:], AF.Tanh)
                        t_tiles[ib] = t
                elif d == 2:
                    for ib in range(IB):
                        w = wp.tile(
                            [P, MACRO], F32R, tag=f"w{ib}",
                            name=f"w{ib}_{rep}_{m}_{oh}_{d}",
                        )
                        nc.scalar.activation(w[:], t_tiles[ib][:], AF.Square)
                        nc.vector.tensor_scalar(w[:], w[:], 2.0, -1.0, OP.mult, OP.add)
                        win[(2, ib)] = w
                elif d >= 3:
                    for ib in range(IB):
                        w = wp.tile(
                            [P, MACRO], F32R, tag=f"w{ib}",
                            name=f"w{ib}_{rep}_{m}_{oh}_{d}",
                        )
                        prev = win[(d - 1, ib)]
                        prev2 = t_tiles[ib] if d == 3 else win[(d - 2, ib)]
                        # w = t * T_{d-1}; w = 2*w - T_{d-2}
                        nc.vector.tensor_mul(w[:], t_tiles[ib][:], prev[:])
                        nc.vector.scalar_tensor_tensor(
                            w[:], w[:], 2.0, prev2[:], OP.mult, OP.subtract
                        )
                        win[(d, ib)] = w

                for ib in range(IB):
                    c2t = c2p.tile(
                        [P, OH], F32R, tag="c2", name=f"c2_{rep}_{m}_{oh}_{d}_{ib}"
                    )
                    r0 = (d * IB + ib) * P
                    nc.sync.dma_start(
                        c2t[:],
                        c2_d[r0:r0 + P, oh * OH:(oh + 1) * OH].bitcast(F32R),
                    )
                    if d == 0:
                        lhs_full = None
                    elif d == 1:
                        lhs_full = t_tiles[ib]
                    else:
                        lhs_full = win[(d, ib)]
                    for bt in range(BT):
                        lhs = (
                            ones[:]
                            if lhs_full is None
                            else lhs_full[:, bt * P:(bt + 1) * P]
                        )
                        nc.tensor.matmul(
                            psum[bt][:],
                            lhs,
                            c2t[:],
                            start=(d == 0 and ib == 0),
                            stop=(d == D1 - 1 and ib == IB - 1),
                        )

            for bt in range(BT):
                ob = op_.tile([P, OH], F32, tag="ob", name=f"ob_{rep}_{m}_{oh}_{bt}")
                nc.vector.tensor_copy(ob[:], psum[bt][:])
                nc.sync.dma_start(
                    y_d[
                        m * MACRO + bt * P:m * MACRO + (bt + 1) * P,
                        oh * OH:(oh + 1) * OH,
                    ],
                    ob[:],
                )


def build_nc(reps=1):
    nc = bacc.Bacc("TRN2", target_bir_lowering=False, debug=False, num_devices=CORES)
    xt_d = nc.dram_tensor("xt", [I, BC], F32, kind="ExternalInput")
    c2_d = nc.dram_tensor("c2", [D1 * I, O], F32, kind="ExternalInput")
    y_d = nc.dram_tensor("y", [BC, O], F32, kind="ExternalOutput")

    with tile.TileContext(nc) as tc:
        with (
            tc.tile_pool(name="xp", bufs=2) as xp,       # x staging
            tc.tile_pool(name="cp", bufs=1) as cp,       # constants
            tc.tile_pool(name="tp", bufs=1) as tp,       # tanh tiles (persist per m)
            tc.tile_pool(name="wp", bufs=3) as wp,       # T_d sliding window
            tc.tile_pool(name="c2p", bufs=6) as c2p,     # coeff stream
            tc.tile_pool(name="op", bufs=4) as op_,      # psum eviction staging
            tc.tile_pool(name="pp", bufs=1, space="PSUM") as pp,
        ):
            # ones tile for the d=0 (T_0 == 1) matmuls; one [P,P] tile reused
            # for every (ib, bt). Built with DVE so its producer dtype is f32r
            # (memset can't write f32r).
            xboot = xp.tile([P, MACRO], F32, tag="xt")
            nc.sync.dma_start(xboot[:, 0:P], xt_d[0:P, 0:P])
            ones = cp.tile([P, P], F32R, tag="ones")
            nc.vector.tensor_scalar(ones[:], xboot[:, 0:P], 0.0, 1.0, OP.mult, OP.add)

            for rep in range(reps):
                _emit(nc, xp, tp, wp, c2p, op_, pp, xt_d, c2_d, y_d, ones, rep)
    nc.compile()
    return nc


def kernel(x: np.ndarray, cheby_coeffs: np.ndarray) -> np.ndarray:
    assert x.shape == (B, I) and cheby_coeffs.shape == (I, O, D1)
    if "nc" not in _CACHE:
        _CACHE["nc"] = build_nc()
    nc = _CACHE["nc"]

    xt = np.ascontiguousarray(x.T.astype(np.float32, copy=False))          # (I, B)
    c2 = np.ascontiguousarray(
        np.transpose(cheby_coeffs.astype(np.float32, copy=False), (2, 0, 1)).reshape(
            D1 * I, O
        )
    )
    in_maps = [
        {"xt": np.ascontiguousarray(xt[:, c * BC:(c + 1) * BC]), "c2": c2}
        for c in range(CORES)
    ]
    global _last_in_maps
    _last_in_maps = in_maps
    res = run_bass_kernel_spmd(nc, in_maps, core_ids=list(range(CORES)))
    return np.concatenate([res.results[c]["y"] for c in range(CORES)], axis=0)
